# revision 17
# baseline (speedup 1.0000x reference)
"""Trainium2 Bass kernel for nn_DotProductAttention_76338748719461.

Attention with a multiplicative mask and softmax over the QUERY axis
(axis=1 of [B, Lq, Lk] scores):

    S[b,q,k]  = (Q[b,q,:] . K[b,k,:]) / 8 + max(log(mask[0,q,k]), F32_MIN)
    A         = softmax(S, axis=q)
    out[b,q,v]= sum_k A[b,q,k] * V[b,k,v]

Key identity: exp(S + log m) = exp(S) * m, so the mask is applied as a
multiply after exp — no log, no additive bias, and mask==0 handled exactly.

v2 design (per NeuronCore; batch data-parallel over 8 cores, 2 per core):
  * ALL layout work happens on host: Q^T (pre-scaled by 1/8), K^T, V in
    f16, and the mask TRANSPOSED to [k, q] in f16. The device does zero
    transposes and zero dtype-conversion DMAs (v1 burned ~33 MB/core of
    HBM traffic and a prep phase on mask cast+transpose).
  * Work in the transposed score orientation S_T[k, q], so the softmax
    reduction (over q) is a free-axis reduction.
  * Per (batch, k-tile j): S_T = KT_j^T @ QT on PE into a [128, 2048]
    PSUM tile (4 banks); one big ACT instr computes E = exp(S_T) f16;
    one DVE tensor_tensor_reduce computes PM = E * mask_T AND the
    softmax denominator D (row sum) in a single pass; Vp = V_j / D.
  * AV with SWAPPED operand roles: stationary = PM chunk [128k, 128q],
    moving = Vp [128k, 64v] => output accumulates directly as
    out[q, v] in PSUM ([128, 16, 64] f32 = 2 banks/batch, double-
    buffered across batches). No output transposes; the result DMAs
    straight from PSUM to DRAM.
  * Steady state is paced by ACT (exp) at ~1.95us per k-tile iteration;
    PE (~1.75us/iter) and DVE (~1.3us/iter) hide underneath.
"""

import os
import numpy as np

B, LQ, LK, D, DV = 16, 2048, 2048, 64, 64
NCORES = 8
BPC = B // NCORES  # batches per core
P = 128
CH = 512  # QK matmul moving chunk (one PSUM bank of fp32)
NT_Q = LQ // P  # 16
NT_K = LK // P  # 16
SCALE = 1.0 / 8.0  # 1/sqrt(64), folded into host-side Q^T prep

MAIN_REPS = int(os.environ.get("MAIN_REPS", "1"))  # repeat body (timing builds)

_CACHED = None


def prep_core_inputs(query, key, value, mask):
    """Host-side layout prep: per-core input dicts for the device binary.

    qt: [BPC, 64, 2048] f16 = (Q/8)^T     kt: [BPC, 64, 2048] f16 = K^T
    v:  [BPC, 2048, 64] f16               mt: [2048, 2048] f16 = mask[0]^T
    """
    query = np.asarray(query, dtype=np.float32)
    key = np.asarray(key, dtype=np.float32)
    value = np.asarray(value, dtype=np.float32)
    mask = np.asarray(mask, dtype=np.float32)

    q16 = (query.transpose(0, 2, 1) * np.float32(SCALE)).astype(np.float16)
    k16 = key.transpose(0, 2, 1).astype(np.float16)
    v16 = value.astype(np.float16)
    m16 = mask[0].T.astype(np.float16)  # astype materializes C-contiguous
    return [
        {
            "qt": np.ascontiguousarray(q16[c * BPC : (c + 1) * BPC]),
            "kt": np.ascontiguousarray(k16[c * BPC : (c + 1) * BPC]),
            "v": np.ascontiguousarray(v16[c * BPC : (c + 1) * BPC]),
            "mt": m16,
        }
        for c in range(NCORES)
    ]


def _emit_av(nc, O, PM, Vp, j):
    """Classic AV: stationary Vp [128, 64], moving PM chunks -> out [v, q].

    Each matmul writes one full PSUM bank ([64, 512] f32), accumulating
    over k-tiles j (start at j==0, stop at j==15)."""
    from concourse.bass import ts

    for c in range(LQ // CH):
        nc.tensor.matmul(
            O[:, ts(c, CH)],
            Vp[:],
            PM[:, ts(c, CH)],
            start=(j == 0),
            stop=(j == NT_K - 1),
        )


def _emit_out(nc, work, o_d, pO, pb, ident, psum_o):
    """Evacuate [v, q] PSUM accumulator: copy to SBUF, PE-transpose back
    to [q, v], copy out, DMA."""
    from concourse.bass import ds

    f32 = pO.dtype
    OT = work.tile([DV, LQ], f32, tag="otmp", bufs=2, name="OT")
    nc.vector.tensor_copy(OT[:], pO[:])
    out_sb = work.tile([P, NT_Q, DV], f32, tag="osb", bufs=2, name="out_sb")
    for g in range(NT_Q // 8):
        tp = psum_o.tile([P, 8 * DV], f32, tag="o", name="tp")
        for u in range(8):
            t = 8 * g + u
            nc.tensor.transpose(
                tp[:, ds(DV * u, DV)],
                OT[:, ds(P * t, P)],
                ident[0:DV, 0:DV],
            )
        nc.vector.tensor_copy(
            out_sb[:, ds(8 * g, 8), :],
            tp[:].rearrange("p (t d) -> p t d", d=DV),
        )
    nc.gpsimd.dma_start(o_d[pb].rearrange("(t p) d -> p t d", p=P), out_sb[:])


def _emit_pass(nc, tc, pools, aps, dts):
    """One full pass: input DMAs + 2 batches x 16 k-tiles + output DMAs."""
    import concourse.mybir as mybir
    from concourse.bass import ds, ts

    qt_d, kt_d, v_d, mt_d, o_d = aps
    f32, f16, AF = dts
    ALU = mybir.AluOpType
    big, psum_s, psum_o, work, small, ident = pools

    mT = big.tile([P, NT_K, LQ], f16, tag="mT", name="mT")
    QT = big.tile([D, BPC, LQ], f16, tag="QT", name="QT")
    KT = big.tile([D, BPC, LK], f16, tag="KT", name="KT")
    Vn = big.tile([P, BPC, NT_K, DV], f16, tag="Vn", name="Vn")

    # Q/K/V on the gpsimd DMA queue (needed first), mask tiles stream on
    # the sync queue in j order (tile j consumed by DVE at ~2us*j).
    nc.gpsimd.dma_start(QT[:], qt_d.rearrange("b d q -> d b q"))
    nc.gpsimd.dma_start(KT[:], kt_d.rearrange("b d q -> d b q"))
    nc.gpsimd.dma_start(Vn[:], v_d.rearrange("b (t p) d -> p b t d", p=P))
    for j in range(NT_K):
        nc.sync.dma_start(mT[:, j, :], mt_d[ds(P * j, P), :])

    pending = None  # (O, PM, Vp, j, b) deferred AV to overlap with next QK
    for b in range(BPC):
        O = psum_o.tile([DV, LQ], f32, tag="o", name=f"O{b}")
        for j in range(NT_K):
            S = psum_s.tile([P, LQ], f32, tag="s", name="S")
            for c in range(LQ // CH):
                nc.tensor.matmul(
                    S[:, ts(c, CH)],
                    KT[:, b, ds(P * j, P)],
                    QT[:, b, ts(c, CH)],
                    start=True,
                    stop=True,
                )
            if pending is not None:
                pO, pPM, pVp, pj, pb = pending
                _emit_av(nc, pO, pPM, pVp, pj)
                if pj == NT_K - 1:
                    _emit_out(nc, work, o_d, pO, pb, ident, psum_o)
            E = work.tile([P, LQ], f16, tag="e", name="E")
            for h in range(2):
                hs = ds(1024 * h, 1024)
                nc.scalar.activation(E[:, hs], S[:, hs], AF.Exp)
            PM = work.tile([P, LQ], f16, tag="pm", name="PM")
            D2 = small.tile([P, 2], f32, tag="d2", name="D2")
            for h in range(2):
                hs = ds(1024 * h, 1024)
                nc.vector.tensor_tensor(
                    PM[:, hs], E[:, hs], mT[:, j, hs], ALU.mult
                )
                nc.vector.tensor_scalar(
                    out=PM[:, hs], in0=PM[:, hs], scalar1=1.0, scalar2=0.0,
                    op0=ALU.mult, op1=ALU.add, accum_out=D2[:, ds(h, 1)],
                )
            Dsum = small.tile([P, 1], f32, tag="d", name="Dsum")
            nc.vector.reduce_sum(Dsum[:], D2[:], axis=mybir.AxisListType.X)
            R = small.tile([P, 1], f32, tag="r", name="R")
            nc.vector.reciprocal(R[:], Dsum[:])
            Vp = small.tile([P, DV], f16, tag="vp", name="Vp")
            nc.vector.tensor_scalar_mul(Vp[:], Vn[:, b, j, :], R[:])
            pending = (O, PM, Vp, j, b)

    pO, pPM, pVp, pj, pb = pending
    _emit_av(nc, pO, pPM, pVp, pj)
    _emit_out(nc, work, o_d, pO, pb, ident, psum_o)


def _build_module():
    import concourse.mybir as mybir
    import concourse.tile as tile
    from concourse import bacc
    from contextlib import ExitStack

    f32 = mybir.dt.float32
    f16 = mybir.dt.float16
    dts = (f32, f16, mybir.ActivationFunctionType)

    nc = bacc.Bacc("TRN2", target_bir_lowering=False, debug=False)
    qt_d = nc.dram_tensor("qt", [BPC, D, LQ], f16, kind="ExternalInput").ap()
    kt_d = nc.dram_tensor("kt", [BPC, D, LK], f16, kind="ExternalInput").ap()
    v_d = nc.dram_tensor("v", [BPC, LK, DV], f16, kind="ExternalInput").ap()
    mt_d = nc.dram_tensor("mt", [LK, LQ], f16, kind="ExternalInput").ap()
    o_d = nc.dram_tensor("o", [BPC, LQ, DV], f32, kind="ExternalOutput").ap()
    aps = (qt_d, kt_d, v_d, mt_d, o_d)

    with tile.TileContext(nc) as tc:
        with ExitStack() as ctx:
            from concourse.masks import make_identity

            big = ctx.enter_context(tc.tile_pool(name="big", bufs=1))
            psum_s = ctx.enter_context(
                tc.tile_pool(name="psum_s", bufs=1, space="PSUM")
            )
            psum_o = ctx.enter_context(
                tc.tile_pool(name="psum_o", bufs=1, space="PSUM")
            )
            work = ctx.enter_context(tc.tile_pool(name="work", bufs=3))
            small = ctx.enter_context(tc.tile_pool(name="small", bufs=3))
            consts = ctx.enter_context(tc.tile_pool(name="consts", bufs=1))
            ident = consts.tile([P, P], f32)
            make_identity(nc, ident)
            pools = (big, psum_s, psum_o, work, small, ident)
            for _ in range(MAIN_REPS):
                _emit_pass(nc, tc, pools, aps, dts)

    nc.compile()
    return nc


def _get_module():
    global _CACHED
    if _CACHED is None:
        _CACHED = _build_module()
    return _CACHED


def kernel(query, key, value, mask, _trace=False):
    from concourse.bass_utils import run_bass_kernel_spmd

    nc = _get_module()
    in_maps = prep_core_inputs(query, key, value, mask)
    res = run_bass_kernel_spmd(
        nc, in_maps, core_ids=list(range(NCORES)), trace=_trace
    )
    out = np.concatenate(
        [res.results[c]["o"] for c in range(NCORES)], axis=0
    ).astype(np.float32)
    if _trace:
        return out, res
    return out


# revision 42
# speedup vs baseline: 1.6602x; 1.6602x over previous
"""Trainium2 Bass kernel for nn_DotProductAttention_76338748719461.

Attention with a multiplicative mask and softmax over the QUERY axis
(axis=1 of [B, Lq, Lk] scores):

    S[b,q,k]  = (Q[b,q,:] . K[b,k,:]) / 8 + max(log(mask[0,q,k]), F32_MIN)
    A         = softmax(S, axis=q)
    out[b,q,v]= sum_k A[b,q,k] * V[b,k,v]

Key identity: exp(S + log m) = exp(S) * m, so the mask is applied as a
multiply after exp — no log, no additive bias, and mask==0 handled exactly.

v2 design (per NeuronCore; batch data-parallel over 8 cores, 2 per core):
  * ALL layout work happens on host: Q^T (pre-scaled by 1/8), K^T, V in
    f16, and the mask TRANSPOSED to [k, q] in f16. The device does zero
    transposes and zero dtype-conversion DMAs (v1 burned ~33 MB/core of
    HBM traffic and a prep phase on mask cast+transpose).
  * Work in the transposed score orientation S_T[k, q], so the softmax
    reduction (over q) is a free-axis reduction.
  * Per (batch, k-tile j): S_T = KT_j^T @ QT on PE into a [128, 2048]
    PSUM tile (4 banks); one big ACT instr computes E = exp(S_T) f16;
    one DVE tensor_tensor_reduce computes PM = E * mask_T AND the
    softmax denominator D (row sum) in a single pass; Vp = V_j / D.
  * AV with SWAPPED operand roles: stationary = PM chunk [128k, 128q],
    moving = Vp [128k, 64v] => output accumulates directly as
    out[q, v] in PSUM ([128, 16, 64] f32 = 2 banks/batch, double-
    buffered across batches). No output transposes; the result DMAs
    straight from PSUM to DRAM.
  * Steady state is paced by ACT (exp) at ~1.95us per k-tile iteration;
    PE (~1.75us/iter) and DVE (~1.3us/iter) hide underneath.
"""

import os
import numpy as np

B, LQ, LK, D, DV = 16, 2048, 2048, 64, 64
NCORES = 8
BPC = B // NCORES  # batches per core
P = 128
CH = 512  # QK matmul moving chunk (one PSUM bank of fp32)
NT_Q = LQ // P  # 16
NT_K = LK // P  # 16
SCALE = 1.0 / 8.0  # 1/sqrt(64), folded into host-side Q^T prep

MAIN_REPS = int(os.environ.get("MAIN_REPS", "1"))  # repeat body (timing builds)

_CACHED = None


def prep_core_inputs(query, key, value, mask):
    """Host-side layout prep: per-core input dicts for the device binary.

    qt: [BPC, 64, 2048] f16 = (Q/8)^T     kt: [BPC, 64, 2048] f16 = K^T
    v:  [BPC, 2048, 64] f16               mt: [2048, 2048] f16 = mask[0]^T
    """
    query = np.asarray(query, dtype=np.float32)
    key = np.asarray(key, dtype=np.float32)
    value = np.asarray(value, dtype=np.float32)
    mask = np.asarray(mask, dtype=np.float32)

    q16 = (query.transpose(0, 2, 1) * np.float32(SCALE)).astype(np.float16)
    k16 = key.transpose(0, 2, 1).astype(np.float16)
    m16 = mask[0].T.astype(np.float16)  # astype materializes C-contiguous
    return [
        {
            "qt": np.ascontiguousarray(q16[c * BPC : (c + 1) * BPC]),
            "kt": np.ascontiguousarray(k16[c * BPC : (c + 1) * BPC]),
            "v": np.ascontiguousarray(value[c * BPC : (c + 1) * BPC]),
            "mt": m16,
        }
        for c in range(NCORES)
    ]


def _emit_av(nc, O, PM, Vp, j):
    """AV with swapped roles: stationary PM chunk [128k, 128q], moving
    Vp [128k, 64v] -> out[q, v] accumulates directly in PSUM (2 banks).

    PSUM accumulation groups are bank-granular (2 KB zero regions, 8
    chunks of [128, 64] f32 per bank): open each bank's group with
    start=True on its first chunk at j==0 (lazy-zeroes the whole bank;
    later j==0 chunks land on pending-zero bytes and replace), close it
    with stop=True on its last chunk at j==15."""
    from concourse.bass import ds

    for t in range(NT_Q):
        nc.tensor.matmul(
            O[:, t, :],
            PM[:, ds(P * t, P)],
            Vp[:],
            start=(j == 0 and t % 8 == 0),
            stop=(j == NT_K - 1 and t % 8 == 7),
            skip_group_check=True,
        )


def _emit_out(nc, work, o_d, pO, pb, ident, psum_o):
    """Evacuate the [q, v]-oriented PSUM accumulator: one DVE copy to
    SBUF (f32 PSUM -> f16, host upcasts), then DMA on the sync queue."""
    import concourse.mybir as mybir

    out_sb = work.tile(
        [P, NT_Q, DV], mybir.dt.float16, tag="osb", bufs=2, name="out_sb"
    )
    nc.vector.tensor_copy(out_sb[:], pO[:])
    nc.sync.dma_start(o_d[pb].rearrange("(t p) d -> p t d", p=P), out_sb[:])


def _emit_pass(nc, tc, pools, aps, dts):
    """One full pass: input DMAs + 2 batches x 16 k-tiles + output DMAs."""
    import concourse.mybir as mybir
    from concourse.bass import ds, ts

    qt_d, kt_d, v_d, mt_d, o_d = aps
    f32, f16, AF = dts
    ALU = mybir.AluOpType
    big, psum_s, psum_o, work, small, ident = pools

    mT = big.tile([P, NT_K, LQ], f16, tag="mT", name="mT")
    QT = big.tile([D, BPC, LQ], f16, tag="QT", name="QT")
    KT = big.tile([D, BPC, LK], f16, tag="KT", name="KT")
    Vn = big.tile([P, BPC, NT_K, DV], f32, tag="Vn", name="Vn")

    # ALL input DMAs go on the sync queue (hardware DGE — the gpsimd
    # queue's software DGE burns ~1.3us of Pool ENGINE time per
    # transfer), hand-ordered so each tile lands just before its first
    # use: batch 0's K/Q first (first QK), then early mask tiles (tile j
    # is consumed at ~2.2us*j), V0 (first Vp), batch 1's K/Q, and the
    # remaining mask tiles, which stay ahead of consumption from there.
    def dma_kqv(b):
        nc.sync.dma_start(KT[:, b, :], kt_d[b])
        nc.sync.dma_start(QT[:, b, :], qt_d[b])

    def dma_v(b):
        nc.sync.dma_start(
            Vn[:, b, :, :], v_d[b].rearrange("(t p) d -> p t d", p=P)
        )

    def dma_m(j):
        nc.sync.dma_start(mT[:, j, :], mt_d[ds(P * j, P), :])

    dma_kqv(0)
    dma_m(0)
    dma_m(1)
    dma_m(2)
    dma_v(0)
    dma_kqv(1)
    dma_m(3)
    dma_v(1)
    for j in range(4, NT_K):
        dma_m(j)

    # Software pipeline at q-half granularity. The two q-halves of the
    # score tile live in SEPARATE PSUM tiles (2 banks each) so dependency
    # tracking is per-half: QK for half t+1 is emitted one ACT-slot ahead
    # of ACT for half t, so the exp stream never waits on the PE.
    # Cross-engine consumers are emitted with a lag so no in-order engine
    # stream ever blocks on a slow producer:
    #   - reciprocal/Vp for unit u are emitted during unit u+1,
    #   - AV matmuls for unit u are emitted during unit u+DEFER (the
    #     denominator chain ACT->DVE->Pool->recip->Vp is ~2.5 units long
    #     and the in-order PE stream would stall on the AV Ldweights).
    # Three rotating half-score tiles (2 banks each; the swapped AV's
    # 2-bank O frees the room): QK for half t+1 only has a WAR against
    # ACT of half t-2, giving the PE a full extra ACT slot of slack.
    HF = LQ // 2
    NS = 3
    S3 = [
        psum_s.tile([P, HF], f32, tag=f"s{h}", name=f"S{h}")
        for h in range(NS)
    ]
    h0s, h1s = ds(0, HF), ds(HF, HF)
    DEFER = 5

    # PE p-state warmup: ~20 dep-free dummy transposes into S3[0] (junk;
    # overwritten by the first QK) run during the input-DMA wait so the
    # 3us ramp to full clock is over by the time real matmuls issue.
    for _ in range(20):
        nc.tensor.transpose(
            S3[0][0:DV, 0:DV], ident[0:DV, 0:DV], ident[0:DV, 0:DV]
        )

    units = [(b, j) for b in range(BPC) for j in range(NT_K)]
    NU = len(units)
    ctx = {}  # u -> dict of tiles
    O_of = {}  # b -> O psum tile

    def emit_qk(t):
        u, h = t // 2, t % 2
        b, j = units[u]
        for c in range(2):
            nc.tensor.matmul(
                S3[t % NS][:, ts(c, CH)],
                KT[:, b, ds(P * j, P)],
                QT[:, b, ds(HF * h + CH * c, CH)],
                start=True,
                stop=True,
            )

    emit_qk(0)
    for t in range(2 * (NU + DEFER) + 2):
        u, h = t // 2, t % 2
        if t + 1 < 2 * NU:
            emit_qk(t + 1)
        if h == 1 and u - DEFER >= 0 and u - DEFER < NU:
            ua = u - DEFER
            ca = ctx[ua]
            ba, ja = units[ua]
            _emit_av(nc, O_of[ba], ca["PM"], ca["Vp"], ja)
            if ja == NT_K - 1:
                _emit_out(nc, work, o_d, O_of[ba], ba, ident, psum_o)
            del ctx[ua]
        if u >= NU:
            continue
        b, j = units[u]
        if h == 0:
            E = work.tile([P, LQ], f16, tag="e", name="E")
            PM = work.tile([P, LQ], f16, tag="pm", name="PM")
            D2 = small.tile([P, 2], f32, tag="d2", name="D2")
            ctx[u] = {"E": E, "PM": PM, "D2": D2}
            if b not in O_of:
                O_of[b] = psum_o.tile(
                    [P, NT_Q, DV], f32, tag="o", name=f"O{b}"
                )
            nc.scalar.activation(E[:, h0s], S3[t % NS][:], AF.Exp)
            # DVE (f16 perf modes beat the un-moded fused op; TensorScalar
            # and free-axis reductions are illegal opcodes on Pool):
            nc.vector.tensor_tensor(
                PM[:, h0s], E[:, h0s], mT[:, j, h0s], ALU.mult
            )
            nc.vector.tensor_scalar(
                out=PM[:, h0s], in0=PM[:, h0s], scalar1=1.0, scalar2=0.0,
                op0=ALU.mult, op1=ALU.add, accum_out=D2[:, ds(0, 1)],
            )
        else:
            E, PM, D2 = ctx[u]["E"], ctx[u]["PM"], ctx[u]["D2"]
            nc.scalar.activation(E[:, h1s], S3[t % NS][:], AF.Exp)
            # lagged Vp for the PREVIOUS unit: one Pool-local
            # normalize_recip (Vp = V / D), so the denominator tail never
            # leaves the Pool engine's in-order stream.
            if u - 1 >= 0 and "Dsum" in ctx.get(u - 1, {}):
                cp = ctx[u - 1]
                bp, jp = units[u - 1]
                Vp = small.tile([P, DV], f16, tag="vp", name="Vp")
                nc.gpsimd.normalize_recip(
                    Vp[:], Vn[:, bp, jp, :], cp["Dsum"][:]
                )
                cp["Vp"] = Vp
            nc.vector.tensor_tensor(
                PM[:, h1s], E[:, h1s], mT[:, j, h1s], ALU.mult
            )
            nc.vector.tensor_scalar(
                out=PM[:, h1s], in0=PM[:, h1s], scalar1=1.0, scalar2=0.0,
                op0=ALU.mult, op1=ALU.add, accum_out=D2[:, ds(1, 1)],
            )
            Dsum = small.tile([P, 1], f32, tag="d", name="Dsum")
            nc.vector.tensor_tensor(
                Dsum[:], D2[:, ds(0, 1)], D2[:, ds(1, 1)], ALU.add
            )
            ctx[u]["Dsum"] = Dsum
            if u == NU - 1:  # no u+1 step will emit our Vp
                Vp = small.tile([P, DV], f16, tag="vp", name="Vp")
                nc.gpsimd.normalize_recip(Vp[:], Vn[:, b, j, :], Dsum[:])
                ctx[u]["Vp"] = Vp


def _build_module():
    import concourse.mybir as mybir
    import concourse.tile as tile
    from concourse import bacc
    from contextlib import ExitStack

    f32 = mybir.dt.float32
    f16 = mybir.dt.float16
    dts = (f32, f16, mybir.ActivationFunctionType)

    nc = bacc.Bacc("TRN2", target_bir_lowering=False, debug=False)
    qt_d = nc.dram_tensor("qt", [BPC, D, LQ], f16, kind="ExternalInput").ap()
    kt_d = nc.dram_tensor("kt", [BPC, D, LK], f16, kind="ExternalInput").ap()
    v_d = nc.dram_tensor("v", [BPC, LK, DV], f32, kind="ExternalInput").ap()
    mt_d = nc.dram_tensor("mt", [LK, LQ], f16, kind="ExternalInput").ap()
    o_d = nc.dram_tensor("o", [BPC, LQ, DV], f16, kind="ExternalOutput").ap()
    aps = (qt_d, kt_d, v_d, mt_d, o_d)

    with tile.TileContext(nc) as tc:
        with ExitStack() as ctx:
            from concourse.masks import make_identity

            big = ctx.enter_context(tc.tile_pool(name="big", bufs=1))
            psum_s = ctx.enter_context(
                tc.tile_pool(name="psum_s", bufs=1, space="PSUM")
            )
            psum_o = ctx.enter_context(
                tc.tile_pool(name="psum_o", bufs=1, space="PSUM")
            )
            work = ctx.enter_context(tc.tile_pool(name="work", bufs=5))
            small = ctx.enter_context(tc.tile_pool(name="small", bufs=5))
            consts = ctx.enter_context(tc.tile_pool(name="consts", bufs=1))
            ident = consts.tile([P, P], f32)
            make_identity(nc, ident)
            pools = (big, psum_s, psum_o, work, small, ident)
            for _ in range(MAIN_REPS):
                _emit_pass(nc, tc, pools, aps, dts)

    nc.compile()
    return nc


def _get_module():
    global _CACHED
    if _CACHED is None:
        _CACHED = _build_module()
    return _CACHED


def kernel(query, key, value, mask, _trace=False):
    from concourse.bass_utils import run_bass_kernel_spmd

    nc = _get_module()
    in_maps = prep_core_inputs(query, key, value, mask)
    res = run_bass_kernel_spmd(
        nc, in_maps, core_ids=list(range(NCORES)), trace=_trace
    )
    out = np.concatenate(
        [res.results[c]["o"] for c in range(NCORES)], axis=0
    ).astype(np.float32)  # device returns f16; upcast to the contract dtype
    if _trace:
        return out, res
    return out


# revision 46
# speedup vs baseline: 1.6739x; 1.0082x over previous
"""Trainium2 Bass kernel for nn_DotProductAttention_76338748719461.

Attention with a multiplicative mask and softmax over the QUERY axis
(axis=1 of [B, Lq, Lk] scores):

    S[b,q,k]  = (Q[b,q,:] . K[b,k,:]) / 8 + max(log(mask[0,q,k]), F32_MIN)
    A         = softmax(S, axis=q)
    out[b,q,v]= sum_k A[b,q,k] * V[b,k,v]

Key identity: exp(S + log m) = exp(S) * m, so the mask is applied as a
multiply after exp — no log, no additive bias, and mask==0 handled exactly.

v2 design (per NeuronCore; batch data-parallel over 8 cores, 2 per core):
  * ALL layout work happens on host: Q^T (pre-scaled by 1/8), K^T, V in
    f16, and the mask TRANSPOSED to [k, q] in f16. The device does zero
    transposes and zero dtype-conversion DMAs (v1 burned ~33 MB/core of
    HBM traffic and a prep phase on mask cast+transpose).
  * Work in the transposed score orientation S_T[k, q], so the softmax
    reduction (over q) is a free-axis reduction.
  * Per (batch, k-tile j): S_T = KT_j^T @ QT on PE into a [128, 2048]
    PSUM tile (4 banks); one big ACT instr computes E = exp(S_T) f16;
    one DVE tensor_tensor_reduce computes PM = E * mask_T AND the
    softmax denominator D (row sum) in a single pass; Vp = V_j / D.
  * AV with SWAPPED operand roles: stationary = PM chunk [128k, 128q],
    moving = Vp [128k, 64v] => output accumulates directly as
    out[q, v] in PSUM ([128, 16, 64] f32 = 2 banks/batch, double-
    buffered across batches). No output transposes; the result DMAs
    straight from PSUM to DRAM.
  * Steady state is paced by ACT (exp) at ~1.95us per k-tile iteration;
    PE (~1.75us/iter) and DVE (~1.3us/iter) hide underneath.
"""

import os
import numpy as np

B, LQ, LK, D, DV = 16, 2048, 2048, 64, 64
NCORES = 8
BPC = B // NCORES  # batches per core
P = 128
CH = 512  # QK matmul moving chunk (one PSUM bank of fp32)
NT_Q = LQ // P  # 16
NT_K = LK // P  # 16
SCALE = 1.0 / 8.0  # 1/sqrt(64), folded into host-side Q^T prep

MAIN_REPS = int(os.environ.get("MAIN_REPS", "1"))  # repeat body (timing builds)

_CACHED = None


def prep_core_inputs(query, key, value, mask):
    """Host-side layout prep: per-core input dicts for the device binary.

    qt: [BPC, 64, 2048] f16 = (Q/8)^T     kt: [BPC, 64, 2048] f16 = K^T
    v:  [BPC, 2048, 64] f16               mt: [2048, 2048] f16 = mask[0]^T
    """
    query = np.asarray(query, dtype=np.float32)
    key = np.asarray(key, dtype=np.float32)
    value = np.asarray(value, dtype=np.float32)
    mask = np.asarray(mask, dtype=np.float32)

    q16 = (query.transpose(0, 2, 1) * np.float32(SCALE)).astype(np.float16)
    k16 = key.transpose(0, 2, 1).astype(np.float16)
    m16 = mask[0].T.astype(np.float16)  # astype materializes C-contiguous
    return [
        {
            "qt": np.ascontiguousarray(q16[c * BPC : (c + 1) * BPC]),
            "kt": np.ascontiguousarray(k16[c * BPC : (c + 1) * BPC]),
            "v": np.ascontiguousarray(value[c * BPC : (c + 1) * BPC]),
            "mt": m16,
        }
        for c in range(NCORES)
    ]


def _emit_av(nc, O, PM, Vp, j):
    """AV with swapped roles: stationary PM chunk [128k, 128q], moving
    Vp [128k, 64v] -> out[q, v] accumulates directly in PSUM (2 banks).

    PSUM accumulation groups are bank-granular (2 KB zero regions, 8
    chunks of [128, 64] f32 per bank): open each bank's group with
    start=True on its first chunk at j==0 (lazy-zeroes the whole bank;
    later j==0 chunks land on pending-zero bytes and replace), close it
    with stop=True on its last chunk at j==15."""
    from concourse.bass import ds

    for t in range(NT_Q):
        nc.tensor.matmul(
            O[:, t, :],
            PM[:, ds(P * t, P)],
            Vp[:],
            start=(j == 0 and t % 8 == 0),
            stop=(j == NT_K - 1 and t % 8 == 7),
            skip_group_check=True,
        )


def _emit_out(nc, work, o_d, pO, pb, ident, psum_o):
    """Evacuate the [q, v]-oriented PSUM accumulator: DVE copy to SBUF
    (f32 PSUM -> f16, host upcasts), then DMA on the sync queue — in two
    halves so the copy and the DMA pipeline."""
    import concourse.mybir as mybir
    from concourse.bass import ds

    out_sb = work.tile(
        [P, NT_Q, DV], mybir.dt.float16, tag="osb", bufs=2, name="out_sb"
    )
    dst = o_d[pb].rearrange("(t p) d -> p t d", p=P)
    half = NT_Q // 2
    for g in range(2):
        gs = ds(g * half, half)
        nc.vector.tensor_copy(out_sb[:, gs, :], pO[:, gs, :])
        nc.sync.dma_start(dst[:, gs, :], out_sb[:, gs, :])


def _emit_pass(nc, tc, pools, aps, dts):
    """One full pass: input DMAs + 2 batches x 16 k-tiles + output DMAs."""
    import concourse.mybir as mybir
    from concourse.bass import ds, ts

    qt_d, kt_d, v_d, mt_d, o_d = aps
    f32, f16, AF = dts
    ALU = mybir.AluOpType
    big, psum_s, psum_o, work, small, ident = pools
    HF0 = LQ // 2

    mT = big.tile([P, NT_K, LQ], f16, tag="mT", name="mT")
    QT = big.tile([D, BPC, LQ], f16, tag="QT", name="QT")
    KT = big.tile([D, BPC, LK], f16, tag="KT", name="KT")
    Vn = big.tile([P, BPC, NT_K, DV], f32, tag="Vn", name="Vn")

    # ALL input DMAs go on the sync queue (hardware DGE — the gpsimd
    # queue's software DGE burns ~1.3us of Pool ENGINE time per
    # transfer), hand-ordered so each tile lands just before its first
    # use: batch 0's K/Q first (first QK), then early mask tiles (tile j
    # is consumed at ~2.2us*j), V0 (first Vp), batch 1's K/Q, and the
    # remaining mask tiles, which stay ahead of consumption from there.
    def dma_v(b):
        nc.sync.dma_start(
            Vn[:, b, :, :], v_d[b].rearrange("(t p) d -> p t d", p=P)
        )

    def dma_m(j):
        nc.sync.dma_start(mT[:, j, :], mt_d[ds(P * j, P), :])

    # just the slices the first QK touches (~150 KB), so the first score
    # matmul can issue ~1us earlier than a full-tile load would allow
    nc.sync.dma_start(KT[:, 0, ds(0, P)], kt_d[0][:, ds(0, P)])
    nc.sync.dma_start(QT[:, 0, ds(0, HF0)], qt_d[0][:, ds(0, HF0)])
    nc.sync.dma_start(KT[:, 0, ds(P, LK - P)], kt_d[0][:, ds(P, LK - P)])
    nc.sync.dma_start(QT[:, 0, ds(HF0, HF0)], qt_d[0][:, ds(HF0, HF0)])
    dma_m(0)
    dma_m(1)
    dma_m(2)
    dma_v(0)
    nc.sync.dma_start(KT[:, 1, :], kt_d[1])
    nc.sync.dma_start(QT[:, 1, :], qt_d[1])
    dma_m(3)
    dma_v(1)
    for j in range(4, NT_K):
        dma_m(j)

    # Software pipeline at q-half granularity. The two q-halves of the
    # score tile live in SEPARATE PSUM tiles (2 banks each) so dependency
    # tracking is per-half: QK for half t+1 is emitted one ACT-slot ahead
    # of ACT for half t, so the exp stream never waits on the PE.
    # Cross-engine consumers are emitted with a lag so no in-order engine
    # stream ever blocks on a slow producer:
    #   - reciprocal/Vp for unit u are emitted during unit u+1,
    #   - AV matmuls for unit u are emitted during unit u+DEFER (the
    #     denominator chain ACT->DVE->Pool->recip->Vp is ~2.5 units long
    #     and the in-order PE stream would stall on the AV Ldweights).
    # Three rotating half-score tiles (2 banks each; the swapped AV's
    # 2-bank O frees the room): QK for half t+1 only has a WAR against
    # ACT of half t-2, giving the PE a full extra ACT slot of slack.
    HF = LQ // 2
    NS = 3
    S3 = [
        psum_s.tile([P, HF], f32, tag=f"s{h}", name=f"S{h}")
        for h in range(NS)
    ]
    h0s, h1s = ds(0, HF), ds(HF, HF)
    DEFER = 5

    # PE p-state warmup: ~20 dep-free dummy transposes into S3[0] (junk;
    # overwritten by the first QK) run during the input-DMA wait so the
    # 3us ramp to full clock is over by the time real matmuls issue.
    for _ in range(20):
        nc.tensor.transpose(
            S3[0][0:DV, 0:DV], ident[0:DV, 0:DV], ident[0:DV, 0:DV]
        )

    units = [(b, j) for b in range(BPC) for j in range(NT_K)]
    NU = len(units)
    ctx = {}  # u -> dict of tiles
    O_of = {}  # b -> O psum tile

    def emit_qk(t):
        u, h = t // 2, t % 2
        b, j = units[u]
        for c in range(2):
            nc.tensor.matmul(
                S3[t % NS][:, ts(c, CH)],
                KT[:, b, ds(P * j, P)],
                QT[:, b, ds(HF * h + CH * c, CH)],
                start=True,
                stop=True,
            )

    emit_qk(0)
    for t in range(2 * (NU + DEFER) + 2):
        u, h = t // 2, t % 2
        if t + 1 < 2 * NU:
            emit_qk(t + 1)
        if h == 1 and u - DEFER >= 0 and u - DEFER < NU:
            ua = u - DEFER
            ca = ctx[ua]
            ba, ja = units[ua]
            _emit_av(nc, O_of[ba], ca["PM"], ca["Vp"], ja)
            if ja == NT_K - 1:
                _emit_out(nc, work, o_d, O_of[ba], ba, ident, psum_o)
            del ctx[ua]
        if u >= NU:
            continue
        b, j = units[u]
        if h == 0:
            E = work.tile([P, LQ], f16, tag="e", name="E")
            PM = work.tile([P, LQ], f16, tag="pm", name="PM")
            D2 = small.tile([P, 2], f32, tag="d2", name="D2")
            ctx[u] = {"E": E, "PM": PM, "D2": D2}
            if b not in O_of:
                O_of[b] = psum_o.tile(
                    [P, NT_Q, DV], f32, tag="o", name=f"O{b}"
                )
            nc.scalar.activation(E[:, h0s], S3[t % NS][:], AF.Exp)
            # DVE (f16 perf modes beat the un-moded fused op; TensorScalar
            # and free-axis reductions are illegal opcodes on Pool):
            nc.vector.tensor_tensor(
                PM[:, h0s], E[:, h0s], mT[:, j, h0s], ALU.mult
            )
            nc.vector.tensor_scalar(
                out=PM[:, h0s], in0=PM[:, h0s], scalar1=1.0, scalar2=0.0,
                op0=ALU.mult, op1=ALU.add, accum_out=D2[:, ds(0, 1)],
            )
        else:
            E, PM, D2 = ctx[u]["E"], ctx[u]["PM"], ctx[u]["D2"]
            nc.scalar.activation(E[:, h1s], S3[t % NS][:], AF.Exp)
            # lagged Vp for the PREVIOUS unit: one Pool-local
            # normalize_recip (Vp = V / D), so the denominator tail never
            # leaves the Pool engine's in-order stream.
            if u - 1 >= 0 and "Dsum" in ctx.get(u - 1, {}):
                cp = ctx[u - 1]
                bp, jp = units[u - 1]
                Vp = small.tile([P, DV], f16, tag="vp", name="Vp")
                nc.gpsimd.normalize_recip(
                    Vp[:], Vn[:, bp, jp, :], cp["Dsum"][:]
                )
                cp["Vp"] = Vp
            nc.vector.tensor_tensor(
                PM[:, h1s], E[:, h1s], mT[:, j, h1s], ALU.mult
            )
            nc.vector.tensor_scalar(
                out=PM[:, h1s], in0=PM[:, h1s], scalar1=1.0, scalar2=0.0,
                op0=ALU.mult, op1=ALU.add, accum_out=D2[:, ds(1, 1)],
            )
            Dsum = small.tile([P, 1], f32, tag="d", name="Dsum")
            nc.vector.tensor_tensor(
                Dsum[:], D2[:, ds(0, 1)], D2[:, ds(1, 1)], ALU.add
            )
            ctx[u]["Dsum"] = Dsum
            if u == NU - 1:  # no u+1 step will emit our Vp
                Vp = small.tile([P, DV], f16, tag="vp", name="Vp")
                nc.gpsimd.normalize_recip(Vp[:], Vn[:, b, j, :], Dsum[:])
                ctx[u]["Vp"] = Vp


def _build_module():
    import concourse.mybir as mybir
    import concourse.tile as tile
    from concourse import bacc
    from contextlib import ExitStack

    f32 = mybir.dt.float32
    f16 = mybir.dt.float16
    dts = (f32, f16, mybir.ActivationFunctionType)

    nc = bacc.Bacc("TRN2", target_bir_lowering=False, debug=False)
    qt_d = nc.dram_tensor("qt", [BPC, D, LQ], f16, kind="ExternalInput").ap()
    kt_d = nc.dram_tensor("kt", [BPC, D, LK], f16, kind="ExternalInput").ap()
    v_d = nc.dram_tensor("v", [BPC, LK, DV], f32, kind="ExternalInput").ap()
    mt_d = nc.dram_tensor("mt", [LK, LQ], f16, kind="ExternalInput").ap()
    o_d = nc.dram_tensor("o", [BPC, LQ, DV], f16, kind="ExternalOutput").ap()
    aps = (qt_d, kt_d, v_d, mt_d, o_d)

    with tile.TileContext(nc) as tc:
        with ExitStack() as ctx:
            from concourse.masks import make_identity

            big = ctx.enter_context(tc.tile_pool(name="big", bufs=1))
            psum_s = ctx.enter_context(
                tc.tile_pool(name="psum_s", bufs=1, space="PSUM")
            )
            psum_o = ctx.enter_context(
                tc.tile_pool(name="psum_o", bufs=1, space="PSUM")
            )
            work = ctx.enter_context(tc.tile_pool(name="work", bufs=8))
            small = ctx.enter_context(tc.tile_pool(name="small", bufs=8))
            consts = ctx.enter_context(tc.tile_pool(name="consts", bufs=1))
            ident = consts.tile([P, P], f32)
            make_identity(nc, ident)
            pools = (big, psum_s, psum_o, work, small, ident)
            for _ in range(MAIN_REPS):
                _emit_pass(nc, tc, pools, aps, dts)

    nc.compile()
    return nc


def _get_module():
    global _CACHED
    if _CACHED is None:
        _CACHED = _build_module()
    return _CACHED


def kernel(query, key, value, mask, _trace=False):
    from concourse.bass_utils import run_bass_kernel_spmd

    nc = _get_module()
    in_maps = prep_core_inputs(query, key, value, mask)
    res = run_bass_kernel_spmd(
        nc, in_maps, core_ids=list(range(NCORES)), trace=_trace
    )
    out = np.concatenate(
        [res.results[c]["o"] for c in range(NCORES)], axis=0
    ).astype(np.float32)  # device returns f16; upcast to the contract dtype
    if _trace:
        return out, res
    return out
